# revision 8
# baseline (speedup 1.0000x reference)
"""Multi-head causal self-attention (B=2, T=2048, D=1024, H=16, Dh=64) on 8
Trainium2 NeuronCores.

Sharding (batch x head-group, Megatron-style within each batch):
  - Core c owns batch b = c//4 and head group g = c%4 (heads 4g..4g+3).
  - x is batch-sliced, host-transposed and host-cast to bf16:
    xT [1024, 2048] per core (4 MB instead of 16 MB replicated fp32).
  - w_qkv column-sharded per group ([1024, 256] per q/k/v, bf16);
    w_proj row-sharded ([256, 1024] bf16). Each core emits a [2048, 1024]
    partial projection for its batch; the host sums the 4 partials per
    batch (plus exact bias fold).

Device-side per core (tokens T=2048 of one batch, 4 heads):
  - qT/kT = W^T x^T via PE with w stationary ([128, 2 ptiles, 2048] bf16).
  - V built directly in [token, feat] layout by making the xT chunk the
    stationary operand (Ldweights are free); PSUM [128 tok, 256 feat]
    evicted in ONE strided DVE copy into V2 [1|Vh0|1|Vh1|1|Vh2|1|Vh3]
    (ones columns give the softmax denominator through the PV matmul).
  - Scores S^T = K Q^T per (head, 256-query superblock) in [keys,
    queries] layout, PSUM groups of 4 key chunks ([128, 1024] fp32 = 2
    banks) so each ScalarE exp pass covers up to 1024 columns.
  - Causal structure: key chunks 0..2sq+1; the odd diagonal chunk is
    computed only for the upper query half (N=128, packed), its fully
    masked half never computed; PV for the low query half skips that
    chunk. Both diagonal masks reduce to the same strictly-lower
    [128,128] triangular -1600 tile folded in as identity.T @ mask
    matmul accumulates (exp then zeroes masked entries exactly).
  - PV with the ones column (N=65) per 128-query sub-block, normalize
    on DVE, heads paired into full [128,128] PE transposes into the
    projection layout, then y_partial = attn_out @ w_proj_slice with
    Pool-engine evictions and SP-queue DMAs out.

Emission software-pipelines: qkv(split s) -> scores(sq=s) ->
attn_output(sq=s-1) -> proj(sq=s-1), with ~3us of PE warmup matmuls
covering the initial weight/x DMA so the cost-model p-state ramp
completes before the first real matmul.
"""

import numpy as np
import ml_dtypes

import concourse.bacc as bacc
import concourse.bass as bass
import concourse.mybir as mybir
import concourse.tile as tile
from concourse.bass_utils import run_bass_kernel_spmd
from concourse.masks import make_identity

N_CORES = 8
B = 2
T = 2048  # tokens per batch (per core)
D = 1024
H = 16
DH = 64
HPC = 4  # heads per core
F = HPC * DH  # 256 attn features per core
P = 128
KC = D // P  # 8 contraction chunks
NKC = T // P  # 16 key chunks
SQ = 256  # superblock query count
NSB = T // SQ  # 8 superblocks
NSPLIT = 8
SPLIT = T // NSPLIT  # 256 tokens per x split
BF = mybir.dt.bfloat16
F32 = mybir.dt.float32

_CACHED_NC = None


def build_nc():
    """Build the per-core Bass program (identical on all 8 cores)."""
    nc = bacc.Bacc("TRN2", target_bir_lowering=False, debug=False, num_devices=N_CORES)

    xT_in = nc.dram_tensor("xT", [D, T], BF, kind="ExternalInput").ap()
    wq_in = nc.dram_tensor("wq", [D, F], BF, kind="ExternalInput").ap()
    wk_in = nc.dram_tensor("wk", [D, F], BF, kind="ExternalInput").ap()
    wv_in = nc.dram_tensor("wv", [D, F], BF, kind="ExternalInput").ap()
    bq_in = nc.dram_tensor("bq", [P, 2], F32, kind="ExternalInput").ap()
    bk_in = nc.dram_tensor("bk", [P, 2], F32, kind="ExternalInput").ap()
    wp_in = nc.dram_tensor("wp", [F, D], BF, kind="ExternalInput").ap()
    mask_in = nc.dram_tensor("mask", [P, P], BF, kind="ExternalInput").ap()
    y_out = nc.dram_tensor("y", [T, D], BF, kind="ExternalOutput").ap()

    with tile.TileContext(nc) as tc:
        with (
            tc.tile_pool(name="const", bufs=1) as const,
            tc.tile_pool(name="qkv", bufs=1) as qkv,
            tc.tile_pool(name="ptp", bufs=8) as ptp,
            tc.tile_pool(name="osml", bufs=4) as osml,
            tc.tile_pool(name="rcp", bufs=8) as rcp,
            tc.tile_pool(name="ystage", bufs=4) as ystage,
            tc.tile_pool(name="ps_mm", bufs=2, space="PSUM") as ps_mm,
            tc.tile_pool(name="ps_st", bufs=3, space="PSUM") as ps_st,
            tc.tile_pool(name="ps_pv", bufs=2, space="PSUM") as ps_pv,
            tc.tile_pool(name="ps_top", bufs=1, space="PSUM") as ps_top,
        ):
            # ---- input staging ----
            # weight chunks land as matmul lhsT tiles [128 d-rows, chunk, feat]
            wq_sb = const.tile([P, KC, F], BF)
            wk_sb = const.tile([P, KC, F], BF)
            wv_sb = const.tile([P, KC, F], BF)
            xT_sb = const.tile([P, KC, T], BF)
            wp_sb = const.tile([P, 2, D], BF)
            bq_sb = const.tile([P, 2], F32)
            bk_sb = const.tile([P, 2], F32)
            mask_sb = const.tile([P, P], BF)

            wq_src = wq_in.rearrange("(c p) f -> p c f", p=P)
            wk_src = wk_in.rearrange("(c p) f -> p c f", p=P)
            wv_src = wv_in.rearrange("(c p) f -> p c f", p=P)
            xT_src = xT_in.rearrange("(c p) t -> p c t", p=P)
            wp_src = wp_in.rearrange("(g p) f -> p g f", p=P)

            # order matters: the SP queue serializes transfers, so the first
            # qkv matmul waits on exactly wq + x split 0.
            nc.sync.dma_start(wq_sb[:], wq_src)
            nc.sync.dma_start(xT_sb[:, :, 0:SPLIT], xT_src[:, :, 0:SPLIT])
            nc.sync.dma_start(wk_sb[:], wk_src)
            nc.sync.dma_start(wv_sb[:], wv_src)
            nc.sync.dma_start(xT_sb[:, :, SPLIT : 2 * SPLIT], xT_src[:, :, SPLIT : 2 * SPLIT])
            nc.sync.dma_start(bq_sb[:], bq_in[:])
            nc.sync.dma_start(bk_sb[:], bk_in[:])
            nc.sync.dma_start(mask_sb[:], mask_in[:])
            nc.sync.dma_start(wp_sb[:], wp_src)
            for s in range(2, NSPLIT):
                nc.sync.dma_start(
                    xT_sb[:, :, s * SPLIT : (s + 1) * SPLIT],
                    xT_src[:, :, s * SPLIT : (s + 1) * SPLIT],
                )

            ident = const.tile([P, P], BF)
            make_identity(nc, ident[:])

            # ---- PE warmup: dependency-free matmuls ramp the p-state model
            # to 2.4 GHz while the first weight/x DMAs stream in ----
            wm = ps_mm.tile([P, 512], F32, name="warm", tag="mm")
            for _ in range(56):
                nc.tensor.matmul(wm[:, 0:P], ident[:], ident[:], start=True, stop=True)

            # ---- persistent activation layout ----
            qT_sb = qkv.tile([P, 2, T], BF)  # [feat-in-ptile, ptile, token]
            kT_sb = qkv.tile([P, 2, T], BF)
            # V2 per key-chunk: [1|Vh0|1|Vh1|1|Vh2|1|Vh3] (4 x 65)
            V2 = qkv.tile([P, NKC, 4, 65], BF)
            nc.vector.memset(V2[:, :, :, 0], 1.0)
            attn_oT = qkv.tile([P, 2, T], BF)

            def qkv_split(s):
                """q/k/V for tokens [SPLIT*s, SPLIT*(s+1))."""
                ts = s * SPLIT
                for w_sb, dst, bias in ((wq_sb, qT_sb, bq_sb), (wk_sb, kT_sb, bk_sb)):
                    for pt in range(2):
                        ps = ps_mm.tile([P, 512], F32, name="psqk", tag="mm")
                        for c in range(KC):
                            nc.tensor.matmul(
                                ps[:, 0:SPLIT],
                                w_sb[:, c, pt * P : (pt + 1) * P],
                                xT_sb[:, c, ts : ts + SPLIT],
                                start=(c == 0),
                                stop=(c == KC - 1),
                            )
                        nc.vector.tensor_scalar(
                            dst[:, pt, ts : ts + SPLIT],
                            ps[:, 0:SPLIT],
                            bias[:, pt : pt + 1],
                            None,
                            op0=mybir.AluOpType.add,
                        )
                # V direct in [token, feat] layout: x chunk stationary
                for tt in range(2):
                    kc = 2 * s + tt
                    ps = ps_mm.tile([P, 512], F32, name="psv", tag="mm")
                    for c in range(KC):
                        nc.tensor.matmul(
                            ps[:, 0:F],
                            xT_sb[:, c, kc * P : (kc + 1) * P],
                            wv_sb[:, c, :],
                            start=(c == 0),
                            stop=(c == KC - 1),
                        )
                    nc.vector.tensor_copy(
                        V2[:, kc, :, 1:65],
                        ps[:, 0:F].rearrange("p (h d) -> p h d", d=DH),
                    )

            def attn_scores(sq):
                """S^T matmuls + exp for one superblock: PE -> ACT -> pt.

                Key chunks 0..2sq+1; PSUM groups of 4 chunks (2 banks); the
                odd diagonal chunk only for the upper query half (packed at
                its 256-col slot's start); masks folded in as matmul
                accumulates."""
                nk = 2 * sq + 2
                pt = [ptp.tile([P, NKC, SQ], BF, name="ptt", tag="pt") for _ in range(HPC)]
                for g in range(0, nk, 2):
                    gn = min(2, nk - g)
                    for h in range(HPC):
                        ptile, hp = h // 2, (h % 2) * DH
                        st = ps_st.tile([P, 512], F32, name="st", tag="st")
                        used = 0
                        mm = []  # (col0, ncols, lhsT, rhs)
                        for j in range(gn):
                            c = g + j
                            odd_diag = c == nk - 1
                            ncols = P if odd_diag else SQ
                            q0 = sq * SQ + (P if odd_diag else 0)
                            col0 = j * SQ
                            mm.append(
                                (
                                    col0,
                                    ncols,
                                    kT_sb[hp : hp + DH, ptile, c * P : (c + 1) * P],
                                    qT_sb[hp : hp + DH, ptile, q0 : sq * SQ + SQ],
                                )
                            )
                            if c >= nk - 2:  # diagonal: fold in causal mask
                                mm.append((col0, P, ident[:], mask_sb[:]))
                            used = col0 + ncols
                        for i, (col0, ncols, lh, rh) in enumerate(mm):
                            # start/stop are per-PSUM-bank flags
                            bank = col0 // 512
                            nc.tensor.matmul(
                                st[:, col0 : col0 + ncols],
                                lh,
                                rh,
                                start=(i == 0 or mm[i - 1][0] // 512 != bank),
                                stop=(i == len(mm) - 1 or mm[i + 1][0] // 512 != bank),
                            )
                        flat = pt[h][:].rearrange("p a b -> p (a b)")
                        nc.scalar.activation(
                            flat[:, g * SQ : g * SQ + used],
                            st[:, 0:used],
                            mybir.ActivationFunctionType.Exp,
                            scale=0.125,
                        )
                return pt

            def attn_output(sq, pt):
                """PV + normalize + paired PE transpose per query sub-block."""
                nk = 2 * sq + 2
                # PV chains packed 4 per PSUM bank (65 fp32 cols each)
                pvt = [
                    ps_pv.tile([P, 4, 65], F32, name="pv", tag="pv") for _ in range(2)
                ]
                work = []
                for h in range(HPC):
                    for qh in (0, 1):
                        # PE stage 1: all chains back-to-back (PE is in-order)
                        nch = nk - 1 if qh == 0 else nk  # low half skips odd diag
                        i = 2 * h + qh
                        pv = pvt[i // 4][:, i % 4, :]
                        for c in range(nch):
                            packed = qh == 1 and c == nk - 1
                            lq = 0 if packed else qh * P
                            nc.tensor.matmul(
                                pv,
                                pt[h][:, c, lq : lq + P],
                                V2[:, c, h, :],
                                start=(c == 0),
                                stop=(c == nch - 1),
                            )
                        work.append((h, qh, pv))
                osbs = {}
                for h, qh, pv in work:
                    # DVE stage: normalize; heads paired into [128,128] tiles
                    pair = h // 2
                    if (pair, qh) not in osbs:
                        osbs[(pair, qh)] = osml.tile([P, P], BF, name="osb")
                    r = rcp.tile([P, 1], F32, name="rr", tag="rr")
                    nc.vector.reciprocal(r[:], pv[:, 0:1])
                    nc.vector.tensor_scalar_mul(
                        osbs[(pair, qh)][:, (h % 2) * DH : (h % 2) * DH + DH],
                        pv[:, 1:65],
                        r[:],
                    )
                top = ps_top.tile([P, 4, P], BF, name="top", tag="top")
                for t, ((pair, qh), osb) in enumerate(osbs.items()):
                    # PE stage 2 + DVE evict into projection layout
                    tk = sq * SQ + qh * P
                    nc.tensor.transpose(top[:, t, :], osb[:], ident[:])
                    nc.vector.tensor_copy(attn_oT[:, pair, tk : tk + P], top[:, t, :])

            def proj(sq):
                """y rows [SQ*sq, SQ*(sq+1)) = attn_out @ w_proj_slice."""
                for tc_ in (2 * sq, 2 * sq + 1):
                    ys = ystage.tile([P, D], BF)
                    for nh in range(2):
                        ps = ps_mm.tile([P, 512], F32, name="psp", tag="mm")
                        for pt_ in range(2):
                            nc.tensor.matmul(
                                ps[:],
                                attn_oT[:, pt_, tc_ * P : (tc_ + 1) * P],
                                wp_sb[:, pt_, nh * 512 : (nh + 1) * 512],
                                start=(pt_ == 0),
                                stop=(pt_ == 1),
                            )
                        nc.vector.tensor_copy(ys[:, nh * 512 : (nh + 1) * 512], ps[:])
                    nc.sync.dma_start(y_out[tc_ * P : (tc_ + 1) * P, :], ys[:])

            # ---- emission: software pipeline at distance 1 ----
            pending = None
            for s in range(NSPLIT):
                qkv_split(s)
                pt = attn_scores(s)
                if pending is not None:
                    psq, ppt = pending
                    attn_output(psq, ppt)
                    proj(psq)
                pending = (s, pt)
            psq, ppt = pending
            attn_output(psq, ppt)
            proj(psq)

    nc.compile()
    return nc


def get_nc():
    global _CACHED_NC
    if _CACHED_NC is None:
        _CACHED_NC = build_nc()
    return _CACHED_NC


def make_in_maps(x, w_qkv, b_qkv, w_proj):
    x = np.asarray(x, dtype=np.float32)
    w_qkv = np.asarray(w_qkv, dtype=np.float32)
    b_qkv = np.asarray(b_qkv, dtype=np.float32)
    w_proj = np.asarray(w_proj, dtype=np.float32)
    bf = ml_dtypes.bfloat16
    xT = [np.ascontiguousarray(x[b].T).astype(bf) for b in range(B)]
    # strictly-lower-triangular -1600 mask (exp(0.125 * -1600) == 0 in fp32)
    kk = np.arange(P)[:, None]
    qq = np.arange(P)[None, :]
    mask = np.where(kk > qq, -1600.0, 0.0).astype(bf)
    in_maps = []
    for c in range(N_CORES):
        b, g = c // 4, c % 4
        lo = g * F
        in_maps.append(
            {
                "xT": xT[b],
                "wq": np.ascontiguousarray(w_qkv[:, lo : lo + F]).astype(bf),
                "wk": np.ascontiguousarray(w_qkv[:, D + lo : D + lo + F]).astype(bf),
                "wv": np.ascontiguousarray(w_qkv[:, 2 * D + lo : 2 * D + lo + F]).astype(bf),
                "bq": np.ascontiguousarray(b_qkv[lo : lo + F].reshape(2, P).T),
                "bk": np.ascontiguousarray(b_qkv[D + lo : D + lo + F].reshape(2, P).T),
                "wp": np.ascontiguousarray(w_proj[lo : lo + F, :]).astype(bf),
                "mask": mask,
            }
        )
    return in_maps


def gather(results, b_qkv, w_proj, b_proj):
    b_qkv = np.asarray(b_qkv, dtype=np.float32)
    w_proj = np.asarray(w_proj, dtype=np.float32)
    b_proj = np.asarray(b_proj, dtype=np.float32)
    y = np.zeros((B, T, D), dtype=np.float32)
    for c in range(N_CORES):
        y[c // 4] += np.asarray(results[c]["y"], dtype=np.float32)
    # exact host-side fold of the v-bias and projection bias: softmax rows
    # sum to 1, so the v-bias passes through attention intact.
    y += b_qkv[2 * D : 3 * D] @ w_proj + b_proj
    return y


def run(x, w_qkv, b_qkv, w_proj, b_proj, trace=False, **spmd_kwargs):
    nc = get_nc()
    in_maps = make_in_maps(x, w_qkv, b_qkv, w_proj)
    res = run_bass_kernel_spmd(
        nc, in_maps, list(range(N_CORES)), trace=trace, **spmd_kwargs
    )
    return gather(res.results, b_qkv, w_proj, b_proj), res


def kernel(x, w_qkv, b_qkv, w_proj, b_proj):
    y, _ = run(x, w_qkv, b_qkv, w_proj, b_proj)
    return y


# revision 15
# speedup vs baseline: 1.0615x; 1.0615x over previous
"""Multi-head causal self-attention (B=2, T=2048, D=1024, H=16, Dh=64) on 8
Trainium2 NeuronCores.

Sharding (batch x head-group, Megatron-style within each batch):
  - Core c owns batch b = c//4 and head group g = c%4 (heads 4g..4g+3).
  - x is batch-sliced, host-transposed and host-cast to bf16:
    xT [1024, 2048] per core (4 MB instead of 16 MB replicated fp32).
  - w_qkv column-sharded per group ([1024, 256] per q/k/v, bf16);
    w_proj row-sharded ([256, 1024] bf16). Each core emits a [2048, 1024]
    partial projection for its batch; the host sums the 4 partials per
    batch (plus exact bias fold).

Device-side per core (tokens T=2048 of one batch, 4 heads):
  - qT/kT = W^T x^T via PE with w stationary ([128, 2 ptiles, 2048] bf16).
  - V built directly in [token, feat] layout by making the xT chunk the
    stationary operand (Ldweights are free); PSUM [128 tok, 256 feat]
    evicted in ONE strided DVE copy into V2 [1|Vh0|1|Vh1|1|Vh2|1|Vh3]
    (ones columns give the softmax denominator through the PV matmul).
  - Scores S^T = K Q^T per (head, 256-query superblock) in [keys,
    queries] layout, PSUM groups of 4 key chunks ([128, 1024] fp32 = 2
    banks) so each ScalarE exp pass covers up to 1024 columns.
  - Causal structure: key chunks 0..2sq+1; the odd diagonal chunk is
    computed only for the upper query half (N=128, packed), its fully
    masked half never computed; PV for the low query half skips that
    chunk. Both diagonal masks reduce to the same strictly-lower
    [128,128] triangular -1600 tile folded in as identity.T @ mask
    matmul accumulates (exp then zeroes masked entries exactly).
  - PV with the ones column (N=65) per 128-query sub-block, normalize
    on DVE, heads paired into full [128,128] PE transposes into the
    projection layout, then y_partial = attn_out @ w_proj_slice with
    Pool-engine evictions and SP-queue DMAs out.

Emission software-pipelines: qkv(split s) -> scores(sq=s) ->
attn_output(sq=s-1) -> proj(sq=s-1), with ~3us of PE warmup matmuls
covering the initial weight/x DMA so the cost-model p-state ramp
completes before the first real matmul.
"""

import numpy as np
import ml_dtypes

import concourse.bacc as bacc
import concourse.bass as bass
import concourse.mybir as mybir
import concourse.tile as tile
from concourse.bass_utils import run_bass_kernel_spmd
from concourse.masks import make_identity

N_CORES = 8
B = 2
T = 2048  # tokens per batch (per core)
D = 1024
H = 16
DH = 64
HPC = 4  # heads per core
F = HPC * DH  # 256 attn features per core
P = 128
KC = D // P  # 8 contraction chunks
NKC = T // P  # 16 key chunks
SQ = 256  # superblock query count
NSB = T // SQ  # 8 superblocks
NSPLIT = 8
SPLIT = T // NSPLIT  # 256 tokens per x split
BF = mybir.dt.bfloat16
F32 = mybir.dt.float32

_CACHED_NC = None


def build_nc():
    """Build the per-core Bass program (identical on all 8 cores)."""
    nc = bacc.Bacc("TRN2", target_bir_lowering=False, debug=False, num_devices=N_CORES)

    xT_in = nc.dram_tensor("xT", [D, T], BF, kind="ExternalInput").ap()
    wq_in = nc.dram_tensor("wq", [D, F], BF, kind="ExternalInput").ap()
    wk_in = nc.dram_tensor("wk", [D, F], BF, kind="ExternalInput").ap()
    wv_in = nc.dram_tensor("wv", [D, F], BF, kind="ExternalInput").ap()
    bq_in = nc.dram_tensor("bq", [P, 2], F32, kind="ExternalInput").ap()
    bk_in = nc.dram_tensor("bk", [P, 2], F32, kind="ExternalInput").ap()
    wp_in = nc.dram_tensor("wp", [F, D], BF, kind="ExternalInput").ap()
    mask_in = nc.dram_tensor("mask", [P, P], BF, kind="ExternalInput").ap()
    y_out = nc.dram_tensor("y", [T, D], BF, kind="ExternalOutput").ap()

    with tile.TileContext(nc) as tc:
        with (
            tc.tile_pool(name="const", bufs=1) as const,
            tc.tile_pool(name="qkv", bufs=1) as qkv,
            tc.tile_pool(name="ptp", bufs=8) as ptp,
            tc.tile_pool(name="osml", bufs=4) as osml,
            tc.tile_pool(name="rcp", bufs=8) as rcp,
            tc.tile_pool(name="ystage", bufs=4) as ystage,
            tc.tile_pool(name="ps_mm", bufs=2, space="PSUM") as ps_mm,
            tc.tile_pool(name="ps_st", bufs=2, space="PSUM") as ps_st,
            tc.tile_pool(name="ps_pv", bufs=2, space="PSUM") as ps_pv,
        ):
            # ---- input staging ----
            # weight chunks land as matmul lhsT tiles [128 d-rows, chunk, feat]
            wq_sb = const.tile([P, KC, F], BF)
            wk_sb = const.tile([P, KC, F], BF)
            wv_sb = const.tile([P, KC, F], BF)
            xT_sb = const.tile([P, KC, T], BF)
            wp_sb = const.tile([P, 2, D], BF)
            bq_sb = const.tile([P, 2], F32)
            bk_sb = const.tile([P, 2], F32)
            mask_sb = const.tile([P, P], BF)

            wq_src = wq_in.rearrange("(c p) f -> p c f", p=P)
            wk_src = wk_in.rearrange("(c p) f -> p c f", p=P)
            wv_src = wv_in.rearrange("(c p) f -> p c f", p=P)
            xT_src = xT_in.rearrange("(c p) t -> p c t", p=P)
            wp_src = wp_in.rearrange("(g p) f -> p g f", p=P)

            # order matters: the SP queue serializes transfers, so the first
            # qkv matmul waits on exactly bias + wq + x split 0 (the tiny
            # bias/mask DMAs go first — the first PSUM eviction reads them).
            nc.sync.dma_start(bq_sb[:], bq_in[:])
            nc.sync.dma_start(bk_sb[:], bk_in[:])
            nc.sync.dma_start(mask_sb[:], mask_in[:])
            nc.sync.dma_start(wq_sb[:], wq_src)
            nc.sync.dma_start(xT_sb[:, :, 0:SPLIT], xT_src[:, :, 0:SPLIT])
            nc.sync.dma_start(wk_sb[:], wk_src)
            nc.sync.dma_start(wv_sb[:], wv_src)
            nc.sync.dma_start(xT_sb[:, :, SPLIT : 2 * SPLIT], xT_src[:, :, SPLIT : 2 * SPLIT])
            nc.sync.dma_start(wp_sb[:], wp_src)
            for s in range(2, NSPLIT):
                nc.sync.dma_start(
                    xT_sb[:, :, s * SPLIT : (s + 1) * SPLIT],
                    xT_src[:, :, s * SPLIT : (s + 1) * SPLIT],
                )

            ident = const.tile([P, P], BF)
            make_identity(nc, ident[:])

            # ---- PE warmup: dependency-free matmuls ramp the p-state model
            # to 2.4 GHz while the first weight/x DMAs stream in ----
            wm = ps_mm.tile([P, 512], F32, name="warm", tag="mm")
            for _ in range(56):
                nc.tensor.matmul(wm[:, 0:P], ident[:], ident[:], start=True, stop=True)

            # ---- persistent activation layout ----
            qT_sb = qkv.tile([P, 2, T], BF)  # [feat-in-ptile, ptile, token]
            kT_sb = qkv.tile([P, 2, T], BF)
            # V2 per key-chunk: [1|Vh0|1|Vh1|1|Vh2|1|Vh3] (4 x 65)
            V2 = qkv.tile([P, NKC, 4, 65], BF)
            nc.vector.memset(V2[:, :, :, 0], 1.0)
            attn_oT = qkv.tile([P, 2, T], BF)

            def qk_split(s):
                """q/k for tokens [SPLIT*s, SPLIT*(s+1)) — before scores(s)."""
                ts = s * SPLIT
                for w_sb, dst, bias in ((wq_sb, qT_sb, bq_sb), (wk_sb, kT_sb, bk_sb)):
                    for pt in range(2):
                        ps = ps_mm.tile([P, 512], F32, name="psqk", tag="mm")
                        for c in range(KC):
                            nc.tensor.matmul(
                                ps[:, 0:SPLIT],
                                w_sb[:, c, pt * P : (pt + 1) * P],
                                xT_sb[:, c, ts : ts + SPLIT],
                                start=(c == 0),
                                stop=(c == KC - 1),
                            )
                        nc.vector.tensor_scalar(
                            dst[:, pt, ts : ts + SPLIT],
                            ps[:, 0:SPLIT],
                            bias[:, pt : pt + 1],
                            None,
                            op0=mybir.AluOpType.add,
                        )

            def v_item(s, tt):
                # V direct in [token, feat] layout: x chunk stationary
                kc = 2 * s + tt
                ps = ps_mm.tile([P, 512], F32, name="psv", tag="mm")
                for c in range(KC):
                    nc.tensor.matmul(
                        ps[:, 0:F],
                        xT_sb[:, c, kc * P : (kc + 1) * P],
                        wv_sb[:, c, :],
                        start=(c == 0),
                        stop=(c == KC - 1),
                    )
                nc.vector.tensor_copy(
                    V2[:, kc, :, 1:65],
                    ps[:, 0:F].rearrange("p (h d) -> p h d", d=DH),
                )

            def score_group(sq, pt, h, g):
                """S^T matmuls + exp for chunks [g, g+4) of head h: PE -> ACT.

                PSUM group of up to 4 key chunks ([128, 1024] fp32 = 2 banks,
                one ScalarE exp pass); the odd diagonal chunk only for the
                upper query half (packed at its 256-col slot's start); masks
                folded in as matmul accumulates."""
                nk = 2 * sq + 2
                gn = min(4, nk - g)
                ptile, hp = h // 2, (h % 2) * DH
                st = ps_st.tile([P, 1024], F32, name="st", tag="st")
                used = 0
                mm = []  # (col0, ncols, lhsT, rhs)
                for j in range(gn):
                    c = g + j
                    odd_diag = c == nk - 1
                    ncols = P if odd_diag else SQ
                    q0 = sq * SQ + (P if odd_diag else 0)
                    col0 = j * SQ
                    mm.append(
                        (
                            col0,
                            ncols,
                            kT_sb[hp : hp + DH, ptile, c * P : (c + 1) * P],
                            qT_sb[hp : hp + DH, ptile, q0 : sq * SQ + SQ],
                        )
                    )
                    if c >= nk - 2:  # diagonal: fold in causal mask
                        mm.append((col0, P, ident[:], mask_sb[:]))
                    used = col0 + ncols
                for i, (col0, ncols, lh, rh) in enumerate(mm):
                    # start/stop are per-PSUM-bank flags
                    bank = col0 // 512
                    nc.tensor.matmul(
                        st[:, col0 : col0 + ncols],
                        lh,
                        rh,
                        start=(i == 0 or mm[i - 1][0] // 512 != bank),
                        stop=(i == len(mm) - 1 or mm[i + 1][0] // 512 != bank),
                    )
                flat = pt[h][:].rearrange("p a b -> p (a b)")
                nc.scalar.activation(
                    flat[:, g * SQ : g * SQ + used],
                    st[:, 0:used],
                    mybir.ActivationFunctionType.Exp,
                    scale=0.125,
                )

            def score_items(sq):
                """h-major so each head's exps finish in sequence and its PV
                chains can start while later heads still exp."""
                nk = 2 * sq + 2
                pt = [ptp.tile([P, NKC, SQ], BF, name="ptt", tag="pt") for _ in range(HPC)]
                items = [
                    (lambda h=h, g=g: score_group(sq, pt, h, g))
                    for h in range(HPC)
                    for g in range(0, nk, 4)
                ]
                return pt, items

            def output_items(sq, pt):
                """PV + normalize + paired PE transpose per query sub-block,
                as fine-grained PE work items for interleaving."""
                nk = 2 * sq + 2
                state = {}

                def chain(h, qh):
                    if "pvt" not in state:
                        # PV chains packed 4 per PSUM bank (65 fp32 cols)
                        state["pvt"] = [
                            ps_pv.tile([P, 4, 65], F32, name="pv", tag="pv")
                            for _ in range(2)
                        ]
                    nch = nk - 1 if qh == 0 else nk  # low half skips odd diag
                    i = 2 * h + qh
                    pv = state["pvt"][i // 4][:, i % 4, :]
                    for c in range(nch):
                        packed = qh == 1 and c == nk - 1
                        lq = 0 if packed else qh * P
                        nc.tensor.matmul(
                            pv,
                            pt[h][:, c, lq : lq + P],
                            V2[:, c, h, :],
                            start=(c == 0),
                            stop=(c == nch - 1),
                        )
                    state[("pv", h, qh)] = pv

                def norm(pair):
                    # DVE: normalize both heads of the pair into one osb tile
                    for h in (2 * pair, 2 * pair + 1):
                        for qh in (0, 1):
                            if ("osb", pair, qh) not in state:
                                state[("osb", pair, qh)] = osml.tile([P, P], BF, name="osb")
                            pv = state[("pv", h, qh)]
                            r = rcp.tile([P, 1], F32, name="rr", tag="rr")
                            nc.vector.reciprocal(r[:], pv[:, 0:1])
                            nc.vector.tensor_scalar_mul(
                                state[("osb", pair, qh)][:, (h % 2) * DH : (h % 2) * DH + DH],
                                pv[:, 1:65],
                                r[:],
                            )

                def transp(pair, qh):
                    if "top" not in state:
                        state["top"] = ps_pv.tile([P, 4, P], BF, name="top", tag="pv")
                    t = 2 * pair + qh
                    tk = sq * SQ + qh * P
                    nc.tensor.transpose(state["top"][:, t, :], state[("osb", pair, qh)][:], ident[:])
                    nc.vector.tensor_copy(
                        attn_oT[:, pair, tk : tk + P], state["top"][:, t, :]
                    )

                return [
                    lambda: chain(0, 0),
                    lambda: chain(0, 1),
                    lambda: chain(1, 0),
                    lambda: (chain(1, 1), norm(0)),
                    lambda: chain(2, 0),
                    lambda: chain(2, 1),
                    lambda: chain(3, 0),
                    lambda: (chain(3, 1), norm(1)),
                    lambda: transp(0, 0),
                    lambda: transp(0, 1),
                    lambda: transp(1, 0),
                    lambda: transp(1, 1),
                ]

            def proj_items(sq):
                """y rows [SQ*sq, SQ*(sq+1)) = attn_out @ w_proj_slice."""
                state = {}

                def half(tc_, nh):
                    if tc_ not in state:
                        state[tc_] = ystage.tile([P, D], BF, name="ys")
                    ys = state[tc_]
                    ps = ps_mm.tile([P, 512], F32, name="psp", tag="mm")
                    for pt_ in range(2):
                        nc.tensor.matmul(
                            ps[:],
                            attn_oT[:, pt_, tc_ * P : (tc_ + 1) * P],
                            wp_sb[:, pt_, nh * 512 : (nh + 1) * 512],
                            start=(pt_ == 0),
                            stop=(pt_ == 1),
                        )
                    nc.vector.tensor_copy(ys[:, nh * 512 : (nh + 1) * 512], ps[:])
                    if nh == 1:
                        nc.sync.dma_start(y_out[tc_ * P : (tc_ + 1) * P, :], ys[:])

                return [
                    (lambda tc_=tc_, nh=nh: half(tc_, nh))
                    for tc_ in (2 * sq, 2 * sq + 1)
                    for nh in range(2)
                ]

            def emit_interleaved(a_items, b_items):
                """Spread b_items (dense PE work) evenly between a_items
                (score groups whose PSUM ring throttles PE to ACT pace)."""
                if not a_items:
                    for f in b_items:
                        f()
                    return
                ratio = len(b_items) / len(a_items)
                acc, bi = 0.0, 0
                for a in a_items:
                    a()
                    acc += ratio
                    while bi < len(b_items) and bi + 1e-9 < acc:
                        b_items[bi]()
                        bi += 1
                while bi < len(b_items):
                    b_items[bi]()
                    bi += 1

            # ---- emission: software pipeline at distance 1; the last two
            # projections are deferred into the drain to cover the final
            # superblocks' exp backlog with dense PE work ----
            DEFER = 2
            pending = None
            for s in range(NSPLIT):
                qk_split(s)
                pt, a_items = score_items(s)
                b_items = [lambda s=s: v_item(s, 0), lambda s=s: v_item(s, 1)]
                if pending is not None:
                    psq, ppt = pending
                    b_items += output_items(psq, ppt)
                    if psq < NSB - 1 - DEFER:
                        b_items += proj_items(psq)
                emit_interleaved(a_items, b_items)
                pending = (s, pt)
            psq, ppt = pending
            tail = output_items(psq, ppt)
            for d in range(DEFER):
                tail += proj_items(NSB - 1 - DEFER + d)
            emit_interleaved([], tail)
            emit_interleaved([], proj_items(psq))

    nc.compile()
    return nc


def get_nc():
    global _CACHED_NC
    if _CACHED_NC is None:
        _CACHED_NC = build_nc()
    return _CACHED_NC


def make_in_maps(x, w_qkv, b_qkv, w_proj):
    x = np.asarray(x, dtype=np.float32)
    w_qkv = np.asarray(w_qkv, dtype=np.float32)
    b_qkv = np.asarray(b_qkv, dtype=np.float32)
    w_proj = np.asarray(w_proj, dtype=np.float32)
    bf = ml_dtypes.bfloat16
    xT = [np.ascontiguousarray(x[b].T).astype(bf) for b in range(B)]
    # strictly-lower-triangular -1600 mask (exp(0.125 * -1600) == 0 in fp32)
    kk = np.arange(P)[:, None]
    qq = np.arange(P)[None, :]
    mask = np.where(kk > qq, -1600.0, 0.0).astype(bf)
    in_maps = []
    for c in range(N_CORES):
        b, g = c // 4, c % 4
        lo = g * F
        in_maps.append(
            {
                "xT": xT[b],
                "wq": np.ascontiguousarray(w_qkv[:, lo : lo + F]).astype(bf),
                "wk": np.ascontiguousarray(w_qkv[:, D + lo : D + lo + F]).astype(bf),
                "wv": np.ascontiguousarray(w_qkv[:, 2 * D + lo : 2 * D + lo + F]).astype(bf),
                "bq": np.ascontiguousarray(b_qkv[lo : lo + F].reshape(2, P).T),
                "bk": np.ascontiguousarray(b_qkv[D + lo : D + lo + F].reshape(2, P).T),
                "wp": np.ascontiguousarray(w_proj[lo : lo + F, :]).astype(bf),
                "mask": mask,
            }
        )
    return in_maps


def gather(results, b_qkv, w_proj, b_proj):
    b_qkv = np.asarray(b_qkv, dtype=np.float32)
    w_proj = np.asarray(w_proj, dtype=np.float32)
    b_proj = np.asarray(b_proj, dtype=np.float32)
    y = np.zeros((B, T, D), dtype=np.float32)
    for c in range(N_CORES):
        y[c // 4] += np.asarray(results[c]["y"], dtype=np.float32)
    # exact host-side fold of the v-bias and projection bias: softmax rows
    # sum to 1, so the v-bias passes through attention intact.
    y += b_qkv[2 * D : 3 * D] @ w_proj + b_proj
    return y


def run(x, w_qkv, b_qkv, w_proj, b_proj, trace=False, **spmd_kwargs):
    nc = get_nc()
    in_maps = make_in_maps(x, w_qkv, b_qkv, w_proj)
    res = run_bass_kernel_spmd(
        nc, in_maps, list(range(N_CORES)), trace=trace, **spmd_kwargs
    )
    return gather(res.results, b_qkv, w_proj, b_proj), res


def kernel(x, w_qkv, b_qkv, w_proj, b_proj):
    y, _ = run(x, w_qkv, b_qkv, w_proj, b_proj)
    return y


# revision 18
# speedup vs baseline: 1.0926x; 1.0293x over previous
"""Multi-head causal self-attention (B=2, T=2048, D=1024, H=16, Dh=64) on 8
Trainium2 NeuronCores.

Sharding (batch x head-group, Megatron-style within each batch):
  - Core c owns batch b = c//4 and head group g = c%4 (heads 4g..4g+3).
  - x is batch-sliced, host-transposed and host-cast to bf16:
    xT [1024, 2048] per core (4 MB instead of 16 MB replicated fp32).
  - w_qkv column-sharded per group ([1024, 256] per q/k/v, bf16);
    w_proj row-sharded ([256, 1024] bf16). Each core emits a [2048, 1024]
    partial projection for its batch; the host sums the 4 partials per
    batch (plus exact bias fold).

Device-side per core (tokens T=2048 of one batch, 4 heads):
  - qT/kT = W^T x^T via PE with w stationary ([128, 2 ptiles, 2048] bf16).
  - V built directly in [token, feat] layout by making the xT chunk the
    stationary operand (Ldweights are free); PSUM [128 tok, 256 feat]
    evicted in ONE strided DVE copy into V2 [1|Vh0|1|Vh1|1|Vh2|1|Vh3]
    (ones columns give the softmax denominator through the PV matmul).
  - Scores S^T = K Q^T per (head, 256-query superblock) in [keys,
    queries] layout, PSUM groups of 4 key chunks ([128, 1024] fp32 = 2
    banks) so each ScalarE exp pass covers up to 1024 columns.
  - Causal structure: key chunks 0..2sq+1; the odd diagonal chunk is
    computed only for the upper query half (N=128, packed), its fully
    masked half never computed; PV for the low query half skips that
    chunk. Both diagonal masks reduce to the same strictly-lower
    [128,128] triangular -1600 tile folded in as identity.T @ mask
    matmul accumulates (exp then zeroes masked entries exactly).
  - PV with the ones column (N=65) per 128-query sub-block, normalize
    on DVE, heads paired into full [128,128] PE transposes into the
    projection layout, then y_partial = attn_out @ w_proj_slice with
    Pool-engine evictions and SP-queue DMAs out.

Emission software-pipelines: qkv(split s) -> scores(sq=s) ->
attn_output(sq=s-1) -> proj(sq=s-1), with ~3us of PE warmup matmuls
covering the initial weight/x DMA so the cost-model p-state ramp
completes before the first real matmul.
"""

import numpy as np
import ml_dtypes

import concourse.bacc as bacc
import concourse.bass as bass
import concourse.mybir as mybir
import concourse.tile as tile
from concourse.bass_utils import run_bass_kernel_spmd
from concourse.masks import make_identity

N_CORES = 8
B = 2
T = 2048  # tokens per batch (per core)
D = 1024
H = 16
DH = 64
HPC = 4  # heads per core
F = HPC * DH  # 256 attn features per core
P = 128
KC = D // P  # 8 contraction chunks
NKC = T // P  # 16 key chunks
SQ = 256  # superblock query count
NSB = T // SQ  # 8 superblocks
NSPLIT = 8
SPLIT = T // NSPLIT  # 256 tokens per x split
BF = mybir.dt.bfloat16
F32 = mybir.dt.float32

_CACHED_NC = None


def build_nc():
    """Build the per-core Bass program (identical on all 8 cores)."""
    nc = bacc.Bacc("TRN2", target_bir_lowering=False, debug=False, num_devices=N_CORES)

    xT_in = nc.dram_tensor("xT", [D, T], BF, kind="ExternalInput").ap()
    wq_in = nc.dram_tensor("wq", [D, F], BF, kind="ExternalInput").ap()
    wk_in = nc.dram_tensor("wk", [D, F], BF, kind="ExternalInput").ap()
    wv_in = nc.dram_tensor("wv", [D, F], BF, kind="ExternalInput").ap()
    bq_in = nc.dram_tensor("bq", [P, 2], F32, kind="ExternalInput").ap()
    bk_in = nc.dram_tensor("bk", [P, 2], F32, kind="ExternalInput").ap()
    wp_in = nc.dram_tensor("wp", [F, D], BF, kind="ExternalInput").ap()
    mask_in = nc.dram_tensor("mask", [P, P], BF, kind="ExternalInput").ap()
    y_out = nc.dram_tensor("y", [T, D], BF, kind="ExternalOutput").ap()

    with tile.TileContext(nc) as tc:
        with (
            tc.tile_pool(name="const", bufs=1) as const,
            tc.tile_pool(name="qkv", bufs=1) as qkv,
            tc.tile_pool(name="ptp", bufs=8) as ptp,
            tc.tile_pool(name="osml", bufs=4) as osml,
            tc.tile_pool(name="rcp", bufs=8) as rcp,
            tc.tile_pool(name="ystage", bufs=4) as ystage,
            tc.tile_pool(name="ps_mm", bufs=2, space="PSUM") as ps_mm,
            tc.tile_pool(name="ps_st", bufs=2, space="PSUM") as ps_st,
            tc.tile_pool(name="ps_pv", bufs=2, space="PSUM") as ps_pv,
        ):
            # ---- input staging ----
            # weight chunks land as matmul lhsT tiles [128 d-rows, chunk, feat]
            wq_sb = const.tile([P, KC, F], BF)
            wk_sb = const.tile([P, KC, F], BF)
            wv_sb = const.tile([P, KC, F], BF)
            xT_sb = const.tile([P, KC, T], BF)
            wp_sb = const.tile([P, 2, D], BF)
            bq_sb = const.tile([P, 2], F32)
            bk_sb = const.tile([P, 2], F32)
            mask_sb = const.tile([P, P], BF)

            wq_src = wq_in.rearrange("(c p) f -> p c f", p=P)
            wk_src = wk_in.rearrange("(c p) f -> p c f", p=P)
            wv_src = wv_in.rearrange("(c p) f -> p c f", p=P)
            xT_src = xT_in.rearrange("(c p) t -> p c t", p=P)
            wp_src = wp_in.rearrange("(g p) f -> p g f", p=P)

            # order matters: the SP queue and the HWDGE serialize transfers,
            # so the first qkv matmul waits on exactly wq + x split 0; the
            # small bias/mask DMAs ride behind (first uses are later).
            nc.sync.dma_start(wq_sb[:], wq_src)
            nc.sync.dma_start(xT_sb[:, :, 0:SPLIT], xT_src[:, :, 0:SPLIT])
            nc.sync.dma_start(bq_sb[:], bq_in[:])
            nc.sync.dma_start(bk_sb[:], bk_in[:])
            nc.sync.dma_start(mask_sb[:], mask_in[:])
            nc.sync.dma_start(wk_sb[:], wk_src)
            nc.sync.dma_start(wv_sb[:], wv_src)
            nc.sync.dma_start(xT_sb[:, :, SPLIT : 2 * SPLIT], xT_src[:, :, SPLIT : 2 * SPLIT])
            nc.sync.dma_start(wp_sb[:], wp_src)
            for s in range(2, NSPLIT):
                nc.sync.dma_start(
                    xT_sb[:, :, s * SPLIT : (s + 1) * SPLIT],
                    xT_src[:, :, s * SPLIT : (s + 1) * SPLIT],
                )

            ident = const.tile([P, P], BF)
            nc.vector.memset(ident[:], 0.0)
            make_identity(nc, ident[:], nomemset=True)

            # ---- PE warmup: dependency-free matmuls ramp the p-state model
            # to 2.4 GHz while the first weight/x DMAs stream in ----
            wm = ps_mm.tile([P, 512], F32, name="warm", tag="mm")
            for _ in range(46):
                nc.tensor.matmul(wm[:, 0:P], ident[:], ident[:], start=True, stop=True)

            # ---- persistent activation layout ----
            qT_sb = qkv.tile([P, 2, T], BF)  # [feat-in-ptile, ptile, token]
            kT_sb = qkv.tile([P, 2, T], BF)
            # V2 per key-chunk: [1|Vh0|1|Vh1|1|Vh2|1|Vh3] (4 x 65)
            V2 = qkv.tile([P, NKC, 4, 65], BF)
            nc.vector.memset(V2[:, :, :, 0], 1.0)
            attn_oT = qkv.tile([P, 2, T], BF)

            def qk_split(s):
                """q/k for tokens [SPLIT*s, SPLIT*(s+1)) — before scores(s)."""
                ts = s * SPLIT
                for w_sb, dst, bias in ((wq_sb, qT_sb, bq_sb), (wk_sb, kT_sb, bk_sb)):
                    for pt in range(2):
                        ps = ps_mm.tile([P, 512], F32, name="psqk", tag="mm")
                        for c in range(KC):
                            nc.tensor.matmul(
                                ps[:, 0:SPLIT],
                                w_sb[:, c, pt * P : (pt + 1) * P],
                                xT_sb[:, c, ts : ts + SPLIT],
                                start=(c == 0),
                                stop=(c == KC - 1),
                            )
                        nc.vector.tensor_scalar(
                            dst[:, pt, ts : ts + SPLIT],
                            ps[:, 0:SPLIT],
                            bias[:, pt : pt + 1],
                            None,
                            op0=mybir.AluOpType.add,
                        )

            def v_item(s, tt):
                # V direct in [token, feat] layout: x chunk stationary
                kc = 2 * s + tt
                ps = ps_mm.tile([P, 512], F32, name="psv", tag="mm")
                for c in range(KC):
                    nc.tensor.matmul(
                        ps[:, 0:F],
                        xT_sb[:, c, kc * P : (kc + 1) * P],
                        wv_sb[:, c, :],
                        start=(c == 0),
                        stop=(c == KC - 1),
                    )
                nc.vector.tensor_copy(
                    V2[:, kc, :, 1:65],
                    ps[:, 0:F].rearrange("p (h d) -> p h d", d=DH),
                )

            def score_group(sq, pt, h, g):
                """S^T matmuls + exp for chunks [g, g+4) of head h: PE -> ACT.

                PSUM group of up to 4 key chunks ([128, 1024] fp32 = 2 banks,
                one ScalarE exp pass); the odd diagonal chunk only for the
                upper query half (packed at its 256-col slot's start); masks
                folded in as matmul accumulates."""
                nk = 2 * sq + 2
                gn = min(4, nk - g)
                ptile, hp = h // 2, (h % 2) * DH
                st = ps_st.tile([P, 1024], F32, name="st", tag="st")
                used = 0
                mm = []  # (col0, ncols, lhsT, rhs)
                for j in range(gn):
                    c = g + j
                    odd_diag = c == nk - 1
                    ncols = P if odd_diag else SQ
                    q0 = sq * SQ + (P if odd_diag else 0)
                    col0 = j * SQ
                    mm.append(
                        (
                            col0,
                            ncols,
                            kT_sb[hp : hp + DH, ptile, c * P : (c + 1) * P],
                            qT_sb[hp : hp + DH, ptile, q0 : sq * SQ + SQ],
                        )
                    )
                    if c >= nk - 2:  # diagonal: fold in causal mask
                        mm.append((col0, P, ident[:], mask_sb[:]))
                    used = col0 + ncols
                for i, (col0, ncols, lh, rh) in enumerate(mm):
                    # start/stop are per-PSUM-bank flags
                    bank = col0 // 512
                    nc.tensor.matmul(
                        st[:, col0 : col0 + ncols],
                        lh,
                        rh,
                        start=(i == 0 or mm[i - 1][0] // 512 != bank),
                        stop=(i == len(mm) - 1 or mm[i + 1][0] // 512 != bank),
                    )
                flat = pt[h][:].rearrange("p a b -> p (a b)")
                nc.scalar.activation(
                    flat[:, g * SQ : g * SQ + used],
                    st[:, 0:used],
                    mybir.ActivationFunctionType.Exp,
                    scale=0.125,
                )

            def score_items(sq):
                """h-major so each head's exps finish in sequence and its PV
                chains can start while later heads still exp."""
                nk = 2 * sq + 2
                pt = [ptp.tile([P, NKC, SQ], BF, name="ptt", tag="pt") for _ in range(HPC)]
                items = [
                    (lambda h=h, g=g: score_group(sq, pt, h, g))
                    for h in range(HPC)
                    for g in range(0, nk, 4)
                ]
                return pt, items

            def output_items(sq, pt):
                """PV + normalize + paired PE transpose per query sub-block,
                as fine-grained PE work items for interleaving."""
                nk = 2 * sq + 2
                state = {}

                def chain(h, qh):
                    if "pvt" not in state:
                        # PV chains packed 4 per PSUM bank (65 fp32 cols)
                        state["pvt"] = [
                            ps_pv.tile([P, 4, 65], F32, name="pv", tag="pv")
                            for _ in range(2)
                        ]
                    nch = nk - 1 if qh == 0 else nk  # low half skips odd diag
                    i = 2 * h + qh
                    pv = state["pvt"][i // 4][:, i % 4, :]
                    for c in range(nch):
                        packed = qh == 1 and c == nk - 1
                        lq = 0 if packed else qh * P
                        nc.tensor.matmul(
                            pv,
                            pt[h][:, c, lq : lq + P],
                            V2[:, c, h, :],
                            start=(c == 0),
                            stop=(c == nch - 1),
                        )
                    state[("pv", h, qh)] = pv

                def norm(pair):
                    # DVE: normalize both heads of the pair into one osb tile
                    for h in (2 * pair, 2 * pair + 1):
                        for qh in (0, 1):
                            if ("osb", pair, qh) not in state:
                                state[("osb", pair, qh)] = osml.tile([P, P], BF, name="osb")
                            pv = state[("pv", h, qh)]
                            r = rcp.tile([P, 1], F32, name="rr", tag="rr")
                            nc.vector.reciprocal(r[:], pv[:, 0:1])
                            nc.vector.tensor_scalar_mul(
                                state[("osb", pair, qh)][:, (h % 2) * DH : (h % 2) * DH + DH],
                                pv[:, 1:65],
                                r[:],
                            )

                def transp(pair, qh):
                    if "top" not in state:
                        state["top"] = ps_pv.tile([P, 4, P], BF, name="top", tag="pv")
                    t = 2 * pair + qh
                    tk = sq * SQ + qh * P
                    nc.tensor.transpose(state["top"][:, t, :], state[("osb", pair, qh)][:], ident[:])
                    nc.vector.tensor_copy(
                        attn_oT[:, pair, tk : tk + P], state["top"][:, t, :]
                    )

                return [
                    lambda: chain(0, 0),
                    lambda: chain(0, 1),
                    lambda: chain(1, 0),
                    lambda: (chain(1, 1), norm(0)),
                    lambda: chain(2, 0),
                    lambda: chain(2, 1),
                    lambda: chain(3, 0),
                    lambda: (chain(3, 1), norm(1)),
                    lambda: transp(0, 0),
                    lambda: transp(0, 1),
                    lambda: transp(1, 0),
                    lambda: transp(1, 1),
                ]

            def proj_items(sq, tail=False):
                """y rows [SQ*sq, SQ*(sq+1)) = attn_out @ w_proj_slice.

                In the drain (tail=True), evictions split across ACT/DVE and
                y DMAs go per column-half so the final eviction -> DMA chain
                is as short as possible."""
                state = {}

                def half(tc_, nh):
                    if tc_ not in state:
                        state[tc_] = ystage.tile([P, D], BF, name="ys")
                    ys = state[tc_]
                    ps = ps_mm.tile([P, 512], F32, name="psp", tag="mm")
                    for pt_ in range(2):
                        nc.tensor.matmul(
                            ps[:],
                            attn_oT[:, pt_, tc_ * P : (tc_ + 1) * P],
                            wp_sb[:, pt_, nh * 512 : (nh + 1) * 512],
                            start=(pt_ == 0),
                            stop=(pt_ == 1),
                        )
                    if tail and nh == 0:
                        nc.scalar.copy(ys[:, 0:512], ps[:])
                    else:
                        nc.vector.tensor_copy(ys[:, nh * 512 : (nh + 1) * 512], ps[:])
                    if tail:
                        nc.sync.dma_start(
                            y_out[tc_ * P : (tc_ + 1) * P, nh * 512 : (nh + 1) * 512],
                            ys[:, nh * 512 : (nh + 1) * 512],
                        )
                    elif nh == 1:
                        nc.sync.dma_start(y_out[tc_ * P : (tc_ + 1) * P, :], ys[:])

                return [
                    (lambda tc_=tc_, nh=nh: half(tc_, nh))
                    for tc_ in (2 * sq, 2 * sq + 1)
                    for nh in range(2)
                ]

            def emit_interleaved(a_items, b_items):
                """Spread b_items (dense PE work) evenly between a_items
                (score groups whose PSUM ring throttles PE to ACT pace)."""
                if not a_items:
                    for f in b_items:
                        f()
                    return
                ratio = len(b_items) / len(a_items)
                acc, bi = 0.0, 0
                for a in a_items:
                    a()
                    acc += ratio
                    while bi < len(b_items) and bi + 1e-9 < acc:
                        b_items[bi]()
                        bi += 1
                while bi < len(b_items):
                    b_items[bi]()
                    bi += 1

            # ---- emission: software pipeline at distance 1. proj(4..6) are
            # deferred so iterations 6-7 carry enough dense PE work to cover
            # the big late superblocks' exp backlog; the drain then runs
            # output(7) (briefly exp-gated) followed by proj(7) ----
            proj_sched = {1: [0], 2: [1], 3: [2], 4: [3], 6: [4], 7: [5, 6]}
            pending = None
            for s in range(NSPLIT):
                qk_split(s)
                pt, a_items = score_items(s)
                b_items = [lambda s=s: v_item(s, 0), lambda s=s: v_item(s, 1)]
                if pending is not None:
                    psq, ppt = pending
                    b_items += output_items(psq, ppt)
                for pq in proj_sched.get(s, []):
                    b_items += proj_items(pq)
                emit_interleaved(a_items, b_items)
                pending = (s, pt)
            psq, ppt = pending
            emit_interleaved([], output_items(psq, ppt))
            emit_interleaved([], proj_items(psq, tail=True))

    nc.compile()
    return nc


def get_nc():
    global _CACHED_NC
    if _CACHED_NC is None:
        _CACHED_NC = build_nc()
    return _CACHED_NC


def make_in_maps(x, w_qkv, b_qkv, w_proj):
    x = np.asarray(x, dtype=np.float32)
    w_qkv = np.asarray(w_qkv, dtype=np.float32)
    b_qkv = np.asarray(b_qkv, dtype=np.float32)
    w_proj = np.asarray(w_proj, dtype=np.float32)
    bf = ml_dtypes.bfloat16
    xT = [np.ascontiguousarray(x[b].T).astype(bf) for b in range(B)]
    # strictly-lower-triangular -1600 mask (exp(0.125 * -1600) == 0 in fp32)
    kk = np.arange(P)[:, None]
    qq = np.arange(P)[None, :]
    mask = np.where(kk > qq, -1600.0, 0.0).astype(bf)
    in_maps = []
    for c in range(N_CORES):
        b, g = c // 4, c % 4
        lo = g * F
        in_maps.append(
            {
                "xT": xT[b],
                "wq": np.ascontiguousarray(w_qkv[:, lo : lo + F]).astype(bf),
                "wk": np.ascontiguousarray(w_qkv[:, D + lo : D + lo + F]).astype(bf),
                "wv": np.ascontiguousarray(w_qkv[:, 2 * D + lo : 2 * D + lo + F]).astype(bf),
                "bq": np.ascontiguousarray(b_qkv[lo : lo + F].reshape(2, P).T),
                "bk": np.ascontiguousarray(b_qkv[D + lo : D + lo + F].reshape(2, P).T),
                "wp": np.ascontiguousarray(w_proj[lo : lo + F, :]).astype(bf),
                "mask": mask,
            }
        )
    return in_maps


def gather(results, b_qkv, w_proj, b_proj):
    b_qkv = np.asarray(b_qkv, dtype=np.float32)
    w_proj = np.asarray(w_proj, dtype=np.float32)
    b_proj = np.asarray(b_proj, dtype=np.float32)
    y = np.zeros((B, T, D), dtype=np.float32)
    for c in range(N_CORES):
        y[c // 4] += np.asarray(results[c]["y"], dtype=np.float32)
    # exact host-side fold of the v-bias and projection bias: softmax rows
    # sum to 1, so the v-bias passes through attention intact.
    y += b_qkv[2 * D : 3 * D] @ w_proj + b_proj
    return y


def run(x, w_qkv, b_qkv, w_proj, b_proj, trace=False, **spmd_kwargs):
    nc = get_nc()
    in_maps = make_in_maps(x, w_qkv, b_qkv, w_proj)
    res = run_bass_kernel_spmd(
        nc, in_maps, list(range(N_CORES)), trace=trace, **spmd_kwargs
    )
    return gather(res.results, b_qkv, w_proj, b_proj), res


def kernel(x, w_qkv, b_qkv, w_proj, b_proj):
    y, _ = run(x, w_qkv, b_qkv, w_proj, b_proj)
    return y
